# revision 1
# baseline (speedup 1.0000x reference)
"""MoE top-1 routing kernel for Trainium2 (8 NeuronCores, expert-F-sharded).

Model (E=8, D=512, F=2048, N=4096):
    logits = x @ Wg + bg; e = argmax(logits)
    y[i] = relu(x[i] @ W1[e] + b1[e]) @ W2[e] + b2[e]

Strategy (v4 — bf16 + quarter-F expert sharding):
- Host computes the gate (f64 matmul + argmax) and routes tokens.
- Each expert's FFN is split into 4 F-quarters (Fs=512). Experts are paired
  (adjacent in sorted-count order) into 4 "slots"; slot s appears on every
  core with the same compile-time token width W_s = max count over the
  slot's two experts. Core j, slot s holds (expert = pair[s][j//4],
  quarter q = j%4) and processes ALL of that expert's tokens against its
  F-quarter. PE work per core = sum_s W_s*Fs/16 cycles — near
  count-independent, so expert imbalance no longer pads every core.
- Everything on the wire is bf16 (halves HBM traffic, enables FWL fast
  weight load; rel-err ~4e-3 vs the 2e-2 gate). PSUM accumulates fp32;
  b1 is applied in the Relu, b2 is added only by the q==0 cell (zeros
  elsewhere), partial y's are summed on the host in fp32.
- y is chunk-major so every output DMA is contiguous per partition.
- Input DMA pieces are issued in exact first-consumption order of the
  software-pipelined emission (st1 of chunk i+1 between st1 and st2 of
  chunk i), with extra-fine first pieces so the first matmul fires early.
- The smallest chunk runs last (short drain tail); its PSUM->SBUF copies
  alternate ACT/DVE and its per-d output DMAs spread across engines.
- A dummy-matmul burst warms the PE clock (HAM) during the DMA head.
"""

import sys

sys.path.insert(0, "/opt/trn_rl_repo")

import numpy as np
import ml_dtypes

BF16 = ml_dtypes.bfloat16
E, D, F, N_CORES = 8, 512, 2048, 8
KD = D // 128      # 4 contraction tiles (stage1) == output d tiles (stage2)
FS = 512           # F-columns per slot (quarter of F)
KQ = FS // 128     # 4
NSLOT = 4

_cache: dict = {}


def _chunks_of(w: int) -> list[int]:
    # split width into <=512-col chunks (PSUM bank limit), evenly
    if w <= 512:
        return [w]
    n = -(-w // 512)
    base = (-(-w // n) + 15) // 16 * 16
    out, rem = [], w
    while rem > 0:
        c = min(base, rem)
        out.append(c)
        rem -= c
    return out


def _chunk_list(widths):
    """Chunk schedule: (slot, lo, cw, xoff, yoff) in execution order; the
    smallest chunk is moved to the end to shorten the drain tail."""
    ch = []
    for s, w in enumerate(widths):
        lo = 0
        for cw in _chunks_of(w):
            ch.append([s, lo, cw])
            lo += cw
    k = min(range(len(ch)), key=lambda i: (ch[i][2], -i))
    ch.append(ch.pop(k))
    off = 0
    out = []
    for s, lo, cw in ch:
        out.append((s, lo, cw, off, off))  # x and y share chunk-major offsets
        off += KD * cw
    return out, off


def _build(widths: tuple[int, ...]):
    import concourse.tile as tile
    import concourse.mybir as mybir
    from concourse import bacc

    f32 = mybir.dt.float32
    bf16 = mybir.dt.bfloat16
    Relu = mybir.ActivationFunctionType.Relu
    Ident = mybir.ActivationFunctionType.Identity

    nc = bacc.Bacc("TRN2", target_bir_lowering=False, debug=False)

    CH, xtot = _chunk_list(widths)
    n_ch = len(CH)

    # Layouts (all [128, *]):
    #   w[p, s*4096 + f*512 + ko*128 + c]        = W1[e][128*ko+p, 512*q + 128*f + c]
    #   w[p, s*4096 + 2048 + d*512 + fo*128 + c] = W2[e][512*q + 128*fo + p, 128*d + c]
    #   x[p, xoff + ko*cw + c]                   = x[tok_{lo+c}, 128*ko + p]
    #   b[p, s*8+f] = b1[e][512*q+128*f+p];  b[p, s*8+4+d] = b2[e][128*d+p] (q==0 else 0)
    #   y[p, yoff + d*cw + c]                    = partial y[tok_{lo+c}, 128*d+p]
    w_d = nc.dram_tensor("w", [128, NSLOT * 4096], bf16, kind="ExternalInput").ap()
    x_d = nc.dram_tensor("x", [128, xtot], bf16, kind="ExternalInput").ap()
    b_d = nc.dram_tensor("b", [128, NSLOT * 8], f32, kind="ExternalInput").ap()
    y_d = nc.dram_tensor("y", [128, xtot], bf16, kind="ExternalOutput").ap()

    # Emission plan: st1(i+1) between st1(i) and st2(i)
    plan = [("st1", 0)]
    for i in range(n_ch):
        if i + 1 < n_ch:
            plan.append(("st1", i + 1))
        plan.append(("st2", i))

    # Input DMA pieces, all on sync (HWDGE) in exact first-consumption order
    # of the software-pipelined emission. ~1024-col (256KB) granularity:
    # each dma_start costs ~0.6us of sequencer issue time, so 23 pieces
    # (~14us of issue) stays just ahead of the ~358GB/s HBM drain while
    # keeping individual completions (and the first matmul) early.
    pieces, seen = [], set()

    def need(t, lo, hi):
        if (t, lo, hi) not in seen:
            seen.add((t, lo, hi))
            pieces.append((t, lo, hi))

    for op, ci in plan:
        s, lo, cw, xoff, yoff = CH[ci]
        wb = s * 4096
        if op == "st1":
            if ci == 0:
                need("w", wb, wb + 1024)
                need("x", xoff, xoff + cw)            # ko0: first MM fires early
                need("x", xoff + cw, xoff + KD * cw)
                need("w", wb + 1024, wb + 2048)
            else:
                need("w", wb, wb + 1024)
                need("x", xoff, xoff + KD * cw)
                need("w", wb + 1024, wb + 2048)
        else:
            need("w", wb + 2048, wb + 3072)
            need("w", wb + 3072, wb + 4096)

    with tile.TileContext(nc) as tc:
        with tc.tile_pool(name="wp", bufs=1) as wp, \
             tc.tile_pool(name="hp", bufs=2) as hp, \
             tc.tile_pool(name="yp", bufs=2) as yp, \
             tc.tile_pool(name="scr", bufs=1) as scr, \
             tc.tile_pool(name="pp", bufs=3, space="PSUM") as pp:

            # --- PE warm-up: dummy matmuls during the DMA head (HAM ramp).
            wrm = scr.tile([128, 256], bf16, name="wrm")
            nc.vector.memset(wrm[:], 0.0)
            wps = pp.tile([128, 256], f32, name="wps", tag="wps", bufs=1)
            for _ in range(30):
                nc.tensor.matmul(wps[:], wrm[:, :128], wrm[:], start=True, stop=True)

            # --- DMA issue (sync = HWDGE), consumption order; b on scalar ---
            bis = wp.tile([128, NSLOT * 8], f32, name="bis")
            nc.scalar.dma_start(bis[:], b_d[:])

            wt = wp.tile([128, NSLOT * 4096], bf16, name="wt")
            xt = wp.tile([128, xtot], bf16, name="xt")
            # w pieces on sync, x pieces on scalar: two HWDGE issue streams
            # in parallel so the head pieces land ~2us earlier.
            for t, lo, hi in pieces:
                if t == "w":
                    nc.sync.dma_start(wt[:, lo:hi], w_d[:, lo:hi])
                else:
                    nc.scalar.dma_start(xt[:, lo:hi], x_d[:, lo:hi])

            # --- compute ---
            hs = {}

            def st1(ci):
                s, lo, cw, xoff, yoff = CH[ci]
                for f in range(KQ):
                    p1 = pp.tile([128, 512], f32, name=f"p1_{ci}_{f}", tag="p1")
                    for ko in range(KD):
                        lhsT = wt[:, s * 4096 + f * 512 + ko * 128:
                                  s * 4096 + f * 512 + ko * 128 + 128]
                        rhs = xt[:, xoff + ko * cw: xoff + (ko + 1) * cw]
                        nc.tensor.matmul(p1[:, :cw], lhsT, rhs,
                                         start=(ko == 0), stop=(ko == KD - 1))
                    h = hp.tile([128, 512], bf16, name=f"h{ci}_{f}", tag=f"h{f}")
                    nc.scalar.activation(h[:, :cw], p1[:, :cw], Relu,
                                         bias=bis[:, s * 8 + f: s * 8 + f + 1])
                    hs[(ci, f)] = h

            def st2(ci, last):
                s, lo, cw, xoff, yoff = CH[ci]
                ys = yp.tile([128, KD * 512], bf16, name=f"ys{ci}", tag="ys")
                for d in range(KD):
                    p2 = pp.tile([128, 512], f32, name=f"p2_{ci}_{d}",
                                 tag=f"p2_{d}", bufs=1)
                    for fo in range(KQ):
                        lhsT = wt[:, s * 4096 + 2048 + d * 512 + fo * 128:
                                  s * 4096 + 2048 + d * 512 + fo * 128 + 128]
                        nc.tensor.matmul(p2[:, :cw], lhsT, hs[(ci, fo)][:, :cw],
                                         start=(fo == 0), stop=(fo == KQ - 1))
                    bcol = bis[:, s * 8 + 4 + d: s * 8 + 4 + d + 1]
                    if last:
                        # alternate ACT/DVE so the drain isn't serialized on one
                        if d % 2 == 0:
                            nc.scalar.activation(ys[:, d * cw:(d + 1) * cw],
                                                 p2[:, :cw], Ident, bias=bcol)
                        else:
                            nc.vector.tensor_scalar_add(ys[:, d * cw:(d + 1) * cw],
                                                        p2[:, :cw], bcol)
                        eng = [nc.gpsimd, nc.scalar, nc.gpsimd, nc.sync][d]
                        eng.dma_start(y_d[:, yoff + d * cw: yoff + (d + 1) * cw],
                                      ys[:, d * cw:(d + 1) * cw])
                    else:
                        nc.vector.tensor_scalar_add(ys[:, d * cw:(d + 1) * cw],
                                                    p2[:, :cw], bcol)
                if not last:
                    nc.scalar.dma_start(y_d[:, yoff: yoff + KD * cw],
                                        ys[:, :KD * cw])

            for op, ci in plan:
                if op == "st1":
                    st1(ci)
                else:
                    st2(ci, last=(ci == n_ch - 1))

    nc.compile()
    return nc


def _get_nc(widths: tuple[int, ...]):
    if widths not in _cache:
        _cache[widths] = _build(widths)
    return _cache[widths]


def _plan(counts):
    """Pair adjacent experts in sorted order into NSLOT slots (minimizes
    sum of per-slot maxima); return (pairs, widths)."""
    order = np.argsort(-counts, kind="stable")
    pairs = [(int(order[2 * s]), int(order[2 * s + 1])) for s in range(NSLOT)]
    widths = tuple(
        (max(int(counts[a]), int(counts[b]), 16) + 15) // 16 * 16
        for a, b in pairs)
    return pairs, widths


def _pack_inputs(x, W1, b1, W2, b2, order, starts, pairs, widths):
    """Build per-core in_maps. Core j, slot s: expert pair[s][j//4], quarter j%4."""
    CH, xtot = _chunk_list(widths)
    xbf = x.astype(BF16)
    toks_of = [order[starts[e]:starts[e + 1]] for e in range(E)]
    in_maps = []
    for j in range(N_CORES):
        q = j % 4
        wcols = np.empty((128, NSLOT * 4096), BF16)
        bcols = np.zeros((128, NSLOT * 8), np.float32)
        xcols = np.zeros((128, xtot), BF16)
        xe_cache = {}
        for s in range(NSLOT):
            e = pairs[s][0] if j < 4 else pairs[s][1]
            # w1 (f-major): [p, f*512 + ko*128 + c]
            w1s = W1[e][:, FS * q: FS * (q + 1)]               # [D, Fs]
            wcols[:, s * 4096: s * 4096 + 2048] = \
                w1s.reshape(KD, 128, KQ, 128).transpose(1, 2, 0, 3).reshape(128, KD * FS)
            # w2 (d-major): [p, d*512 + fo*128 + c]
            w2s = W2[e][FS * q: FS * (q + 1), :]               # [Fs, D]
            wcols[:, s * 4096 + 2048: s * 4096 + 4096] = \
                w2s.reshape(KQ, 128, KD, 128).transpose(1, 2, 0, 3).reshape(128, KQ * D)
            bcols[:, s * 8: s * 8 + KQ] = b1[e][FS * q: FS * (q + 1)].reshape(KQ, 128).T
            if q == 0:
                bcols[:, s * 8 + 4: s * 8 + 8] = b2[e].reshape(KD, 128).T
            toks = toks_of[e]
            xe = np.zeros((widths[s], D), BF16)
            xe[:len(toks)] = xbf[toks]
            xe_cache[s] = xe.T                                  # [D, W]
        for s, lo, cw, xoff, yoff in CH:
            xcols[:, xoff: xoff + KD * cw] = \
                xe_cache[s][:, lo:lo + cw].reshape(KD, 128, cw) \
                .transpose(1, 0, 2).reshape(128, KD * cw)
        in_maps.append({
            "w": np.ascontiguousarray(wcols),
            "x": np.ascontiguousarray(xcols),
            "b": bcols,
        })
    return in_maps, toks_of


def kernel(x, Wg, bg, W1, b1, W2, b2):
    from concourse.bass_utils import run_bass_kernel_spmd

    x = np.asarray(x, dtype=np.float32)
    n_tok = x.shape[0]

    # host gate in f64: the mathematically-true argmax
    logits = x.astype(np.float64) @ np.asarray(Wg, np.float64) + np.asarray(bg, np.float64)
    idx = logits.argmax(1)

    counts = np.bincount(idx, minlength=E)
    order = np.argsort(idx, kind="stable")
    starts = np.zeros(E + 1, np.int64)
    starts[1:] = np.cumsum(counts)

    pairs, widths = _plan(counts)

    W1 = np.asarray(W1, np.float32)
    W2 = np.asarray(W2, np.float32)
    b1 = np.asarray(b1, np.float32)
    b2 = np.asarray(b2, np.float32)

    in_maps, toks_of = _pack_inputs(x, W1, b1, W2, b2, order, starts, pairs, widths)
    nc = _get_nc(widths)
    res = run_bass_kernel_spmd(nc, in_maps, core_ids=list(range(N_CORES)))

    CH, xtot = _chunk_list(widths)
    out = np.zeros((n_tok, D), np.float32)
    for j in range(N_CORES):
        yv = res.results[j]["y"]
        for s, lo, cw, xoff, yoff in CH:
            e = pairs[s][0] if j < 4 else pairs[s][1]
            toks = toks_of[e]
            seg = toks[lo:lo + cw]
            if len(seg) == 0:
                continue
            blk = yv[:, yoff: yoff + KD * cw].astype(np.float32) \
                .reshape(128, KD, cw).transpose(2, 1, 0).reshape(cw, D)
            out[seg] += blk[:len(seg)]
    return out



# revision 3
# speedup vs baseline: 1.0353x; 1.0353x over previous
"""MoE top-1 routing kernel for Trainium2 (8 NeuronCores, expert-half-F sharded).

Model (E=8, D=512, F=2048, N=4096):
    logits = x @ Wg + bg; e = argmax(logits)
    y[i] = relu(x[i] @ W1[e] + b1[e]) @ W2[e] + b2[e]

Strategy (v5 — M=2 half-F cells + queue re-routing):
- Host computes the gate (f64 matmul + argmax) and routes tokens.
- Each expert's FFN is split into 2 F-halves (Fh=1024). Sorted by count,
  experts order[0..3] fill cell-slot A (width W0 = max of their counts),
  order[4..7] fill slot B (width W1). Core j holds (expert order[j%4],
  half j//4) in slot A and (expert order[4+j%4], half j//4) in slot B.
  All cores share the same compile-time widths (W0, W1) -> one SPMD
  program; per-core HBM = 4MiB weights + ~1MiB x + ~1MiB y (vs 8.15MiB
  for quarter-F), giving DMA 2x slack over the PE at the ridge.
- Everything on the wire is bf16; PSUM accumulates fp32; b1 applied in
  the Relu, b2 added only by half-0 cores (zeros elsewhere); the two
  half partial y's are summed on the host in fp32.
- Queue routing keeps the scalar/vector queues free of bulk DMA issue:
  w pieces on sync, x(chunk0) on vector, b on gpsimd, later x pieces on
  scalar interleaved one chunk ahead of their consumption. Element ops
  split by parity: Relu/bias-add alternate scalar ACTIVATE and vector
  dual-op TENSOR_SCALAR so neither queue stalls the PSUM-pool rotation.
- y is chunk-major; non-last chunks go out as one contiguous DMA on
  sync; the last (smallest) chunk drains per-d across engines.
- A short dummy-matmul burst (6) warms the PE clock until chunk-0 data
  lands; real matmuls then keep the HAM ramp going.
"""

import sys

sys.path.insert(0, "/opt/trn_rl_repo")

import numpy as np
import ml_dtypes

BF16 = ml_dtypes.bfloat16
E, D, F, N_CORES = 8, 512, 2048, 8
KD = D // 128       # 4 contraction tiles (stage1) == output d tiles (stage2)
FH = 1024           # F-columns per cell (half of F)
KF = FH // 128      # 8 f tiles per cell
NCELL = 2
CELLW = 2 * KF * 512  # 8192 w cols per cell (w1 4096 + w2 4096)

_cache: dict = {}


def _chunks_of(w: int) -> list[int]:
    # split width into <=288-col chunks (half-PSUM-bank granularity), evenly
    n = -(-w // 288)
    base = (-(-w // n) + 15) // 16 * 16
    out, rem = [], w
    while rem > 0:
        c = min(base, rem)
        out.append(c)
        rem -= c
    return out


def _chunk_list(widths):
    """Chunk schedule: (cell, lo, cw, off) in execution order; the smallest
    chunk is moved to the end to shorten the drain tail."""
    ch = []
    for s, w in enumerate(widths):
        lo = 0
        for cw in _chunks_of(w):
            ch.append([s, lo, cw])
            lo += cw
    k = min(range(len(ch)), key=lambda i: (ch[i][2], -i))
    ch.append(ch.pop(k))
    off = 0
    out = []
    for s, lo, cw in ch:
        out.append((s, lo, cw, off))  # x and y share chunk-major offsets
        off += KD * cw
    return out, off


def _build(widths: tuple[int, ...]):
    import concourse.tile as tile
    import concourse.mybir as mybir
    from concourse import bacc

    f32 = mybir.dt.float32
    bf16 = mybir.dt.bfloat16
    Relu = mybir.ActivationFunctionType.Relu
    Ident = mybir.ActivationFunctionType.Identity
    Add = mybir.AluOpType.add
    Max = mybir.AluOpType.max

    nc = bacc.Bacc("TRN2", target_bir_lowering=False, debug=False)

    CH, xtot = _chunk_list(widths)
    n_ch = len(CH)

    # Layouts (all [128, *]):
    #   w[p, s*8192 + f*512 + ko*128 + c]        = W1[e][128*ko+p, 1024*h + 128*f + c]
    #   w[p, s*8192 + 4096 + d*1024 + fo*128 + c] = W2[e][1024*h + 128*fo + p, 128*d + c]
    #   x[p, xoff + ko*cw + c]                   = x[tok_{lo+c}, 128*ko + p]
    #   b[p, s*12+f] = b1[e][1024*h+128*f+p];  b[p, s*12+8+d] = b2[e][128*d+p] (h==0 else 0)
    #   y[p, yoff + d*cw + c]                    = partial y[tok_{lo+c}, 128*d+p]
    w_d = nc.dram_tensor("w", [128, NCELL * CELLW], bf16, kind="ExternalInput").ap()
    x_d = nc.dram_tensor("x", [128, xtot], bf16, kind="ExternalInput").ap()
    b_d = nc.dram_tensor("b", [128, NCELL * 12], f32, kind="ExternalInput").ap()
    y_d = nc.dram_tensor("y", [128, xtot], bf16, kind="ExternalOutput").ap()

    # Emission plan: st1(i+1) between st1(i) and st2(i)
    plan = [("st1", 0)]
    for i in range(n_ch):
        if i + 1 < n_ch:
            plan.append(("st1", i + 1))
        plan.append(("st2", i))

    # w pieces (sync queue), in consumption order. Cell 0's w1 is split fine
    # so the first matmul group fires as early as possible; everything later
    # uses 2048-col (512KB) pieces to amortize the ~0.65us issue cost.
    wpieces = []
    seen = set()

    def need_w(kind, s):
        if (kind, s) in seen:
            return
        seen.add((kind, s))
        base = s * CELLW + (0 if kind == "w1" else KF * 512)
        if kind == "w1" and s == 0:
            wpieces.append(("head", base, base + 512))
            wpieces.append(("head", base + 512, base + 2048))
            wpieces.append(("head", base + 2048, base + 4096))
        else:
            wpieces.append(("late", base, base + 2048))
            wpieces.append(("late", base + 2048, base + 4096))

    for op, ci in plan:
        s = CH[ci][0]
        if op == "st1":
            need_w("w1", s)
        else:
            need_w("w2", s)

    with tile.TileContext(nc) as tc:
        with tc.tile_pool(name="wp", bufs=1) as wp, \
             tc.tile_pool(name="hp", bufs=2) as hp, \
             tc.tile_pool(name="yp", bufs=2) as yp, \
             tc.tile_pool(name="scr", bufs=1) as scr, \
             tc.tile_pool(name="pp", bufs=3, space="PSUM") as pp:

            # --- PE warm-up: dummy matmuls bridge until chunk-0 data lands
            # (HAM clock ramp needs sustained PE activity).
            wrm = scr.tile([128, 256], bf16, name="wrm")
            nc.gpsimd.memset(wrm[:], 0.0)
            wps = pp.tile([128, 256], f32, name="wps", tag="wps", bufs=1)
            for _ in range(6):
                nc.tensor.matmul(wps[:], wrm[:, :128], wrm[:], start=True, stop=True)

            wt = wp.tile([128, NCELL * CELLW], bf16, name="wt")
            xt = wp.tile([128, xtot], bf16, name="xt")
            bis = wp.tile([128, NCELL * 12], f32, name="bis")

            # --- head DMA issue ---
            # sync: w pieces; gpsimd: chunk-0 x ko0 + b (small, SWDGE ok);
            # scalar: chunk-0 x rest (runs right after its auto act-table load)
            for _, lo, hi in wpieces:
                nc.sync.dma_start(wt[:, lo:hi], w_d[:, lo:hi])
            s0, lo0, cw0, off0 = CH[0]
            nc.gpsimd.dma_start(xt[:, off0: off0 + cw0], x_d[:, off0: off0 + cw0])
            nc.gpsimd.dma_start(bis[:], b_d[:])
            nc.scalar.dma_start(xt[:, off0 + cw0: off0 + KD * cw0],
                                x_d[:, off0 + cw0: off0 + KD * cw0])

            # --- compute ---
            hs = {}
            xseen = {0}

            def xload(ci):
                if ci in xseen or ci >= n_ch:
                    return
                xseen.add(ci)
                _, _, cw, off = CH[ci]
                nc.scalar.dma_start(xt[:, off: off + KD * cw],
                                    x_d[:, off: off + KD * cw])

            def st1(ci):
                xload(ci + 1)  # scalar issues next chunk's x one chunk ahead
                s, lo, cw, off = CH[ci]
                for f in range(KF):
                    p1 = pp.tile([128, 512], f32, name=f"p1_{ci}_{f}", tag="p1")
                    for ko in range(KD):
                        lhsT = wt[:, s * CELLW + f * 512 + ko * 128:
                                  s * CELLW + f * 512 + ko * 128 + 128]
                        rhs = xt[:, off + ko * cw: off + (ko + 1) * cw]
                        nc.tensor.matmul(p1[:, :cw], lhsT, rhs,
                                         start=(ko == 0), stop=(ko == KD - 1))
                    h = hp.tile([128, 512], bf16, name=f"h{ci}_{f}", tag=f"h{f}")
                    bcol = bis[:, s * 12 + f: s * 12 + f + 1]
                    if f % 2 == 0:
                        nc.scalar.activation(h[:, :cw], p1[:, :cw], Relu, bias=bcol)
                    else:
                        nc.vector.tensor_scalar(h[:, :cw], p1[:, :cw], bcol, 0.0,
                                                Add, Max)
                    hs[(ci, f)] = h

            def st2(ci, last):
                s, lo, cw, off = CH[ci]
                ys = yp.tile([128, KD * 512], bf16, name=f"ys{ci}", tag="ys")
                for d in range(KD):
                    p2 = pp.tile([128, 512], f32, name=f"p2_{ci}_{d}",
                                 tag=f"p2_{d}", bufs=1)
                    for fo in range(KF):
                        lhsT = wt[:, s * CELLW + 4096 + d * 1024 + fo * 128:
                                  s * CELLW + 4096 + d * 1024 + fo * 128 + 128]
                        nc.tensor.matmul(p2[:, :cw], lhsT, hs[(ci, fo)][:, :cw],
                                         start=(fo == 0), stop=(fo == KF - 1))
                    bcol = bis[:, s * 12 + 8 + d: s * 12 + 8 + d + 1]
                    if d % 2 == 0:
                        nc.scalar.activation(ys[:, d * cw:(d + 1) * cw],
                                             p2[:, :cw], Ident, bias=bcol)
                    else:
                        nc.vector.tensor_scalar_add(ys[:, d * cw:(d + 1) * cw],
                                                    p2[:, :cw], bcol)
                    if last:
                        eng = [nc.gpsimd, nc.scalar, nc.gpsimd, nc.sync][d]
                        eng.dma_start(y_d[:, off + d * cw: off + (d + 1) * cw],
                                      ys[:, d * cw:(d + 1) * cw])
                if not last:
                    nc.sync.dma_start(y_d[:, off: off + KD * cw],
                                      ys[:, :KD * cw])

            for op, ci in plan:
                if op == "st1":
                    st1(ci)
                else:
                    st2(ci, last=(ci == n_ch - 1))

    nc.compile()
    return nc


def _get_nc(widths: tuple[int, ...]):
    if widths not in _cache:
        _cache[widths] = _build(widths)
    return _cache[widths]


def _plan(counts):
    """Experts sorted by count; order[0..3] fill slot A (width = max of their
    counts), order[4..7] slot B. Returns (order, widths)."""
    order = np.argsort(-counts, kind="stable")
    w0 = max(int(counts[order[0]]), 16)
    w1 = max(int(counts[order[4]]), 16)
    widths = ((w0 + 15) // 16 * 16, (w1 + 15) // 16 * 16)
    return order, widths


def _pack_inputs(x, W1, b1, W2, b2, order, starts, tok_order, widths):
    """Build per-core in_maps. Core j: slot s in {0,1}: expert
    order[4*s + j%4], half j//4."""
    CH, xtot = _chunk_list(widths)
    xbf = x.astype(BF16)
    toks_of = [tok_order[starts[e]:starts[e + 1]] for e in range(E)]
    # xe (token-major padded x, transposed) is identical for both halves ->
    # build once per slot-expert.
    xe_T = {}
    for s in range(NCELL):
        for p in range(4):
            e = int(order[4 * s + p])
            toks = toks_of[e]
            xe = np.zeros((widths[s], D), BF16)
            xe[:len(toks)] = xbf[toks]
            xe_T[e] = xe.T  # [D, W]
    in_maps = []
    for j in range(N_CORES):
        h = j // 4
        wcols = np.empty((128, NCELL * CELLW), BF16)
        bcols = np.zeros((128, NCELL * 12), np.float32)
        xcols = np.zeros((128, xtot), BF16)
        for s in range(NCELL):
            e = int(order[4 * s + j % 4])
            w1s = W1[e][:, FH * h: FH * (h + 1)]               # [D, Fh]
            wcols[:, s * CELLW: s * CELLW + 4096] = \
                w1s.reshape(KD, 128, KF, 128).transpose(1, 2, 0, 3).reshape(128, KF * 512)
            w2s = W2[e][FH * h: FH * (h + 1), :]               # [Fh, D]
            wcols[:, s * CELLW + 4096: s * CELLW + 8192] = \
                w2s.reshape(KF, 128, KD, 128).transpose(1, 2, 0, 3).reshape(128, KD * 1024)
            bcols[:, s * 12: s * 12 + KF] = \
                b1[e][FH * h: FH * (h + 1)].reshape(KF, 128).T
            if h == 0:
                bcols[:, s * 12 + 8: s * 12 + 12] = b2[e].reshape(KD, 128).T
        for s, lo, cw, off in CH:
            e = int(order[4 * s + j % 4])
            xcols[:, off: off + KD * cw] = \
                xe_T[e][:, lo:lo + cw].reshape(KD, 128, cw) \
                .transpose(1, 0, 2).reshape(128, KD * cw)
        in_maps.append({
            "w": np.ascontiguousarray(wcols),
            "x": np.ascontiguousarray(xcols),
            "b": bcols,
        })
    return in_maps, toks_of


def kernel(x, Wg, bg, W1, b1, W2, b2):
    from concourse.bass_utils import run_bass_kernel_spmd

    x = np.asarray(x, dtype=np.float32)
    n_tok = x.shape[0]

    # host gate in f64: the mathematically-true argmax
    logits = x.astype(np.float64) @ np.asarray(Wg, np.float64) + np.asarray(bg, np.float64)
    idx = logits.argmax(1)

    counts = np.bincount(idx, minlength=E)
    tok_order = np.argsort(idx, kind="stable")
    starts = np.zeros(E + 1, np.int64)
    starts[1:] = np.cumsum(counts)

    order, widths = _plan(counts)

    W1 = np.asarray(W1, np.float32)
    W2 = np.asarray(W2, np.float32)
    b1 = np.asarray(b1, np.float32)
    b2 = np.asarray(b2, np.float32)

    in_maps, toks_of = _pack_inputs(x, W1, b1, W2, b2, order, starts, tok_order, widths)
    nc = _get_nc(widths)
    res = run_bass_kernel_spmd(nc, in_maps, core_ids=list(range(N_CORES)))

    CH, xtot = _chunk_list(widths)
    out = np.zeros((n_tok, D), np.float32)
    for j in range(N_CORES):
        yv = res.results[j]["y"]
        for s, lo, cw, off in CH:
            e = int(order[4 * s + j % 4])
            toks = toks_of[e]
            seg = toks[lo:lo + cw]
            if len(seg) == 0:
                continue
            blk = yv[:, off: off + KD * cw].astype(np.float32) \
                .reshape(128, KD, cw).transpose(2, 1, 0).reshape(cw, D)
            out[seg] += blk[:len(seg)]
    return out


# revision 4
# speedup vs baseline: 1.1006x; 1.0631x over previous
"""MoE top-1 routing kernel for Trainium2 (8 NeuronCores, expert-F-sharded).

Model (E=8, D=512, F=2048, N=4096):
    logits = x @ Wg + bg; e = argmax(logits)
    y[i] = relu(x[i] @ W1[e] + b1[e]) @ W2[e] + b2[e]

Strategy (v6 — quarter-F expert pairing + queue re-routing):
- Host computes the gate (f64 matmul + argmax) and routes tokens.
- Each expert's FFN is split into 4 F-quarters (Fs=512). Experts are paired
  (adjacent in sorted-count order) into 4 "slots"; slot s appears on every
  core with the same compile-time token width W_s = max count over the
  slot's two experts. Core j, slot s holds (expert = pair[s][j//4],
  quarter q = j%4) and processes ALL of that expert's tokens against its
  F-quarter. PE work per core = 32*sum_s W_s cycles; adjacent-pairing
  minimizes sum of pair maxima, so expert imbalance costs only ~4%.
- Everything on the wire is bf16; PSUM accumulates fp32; b1 is applied in
  the Relu, b2 is added only by the q==0 cell (zeros elsewhere), partial
  y's are summed on the host in fp32.
- Queue routing (the v5 lesson): the scalar/vector queues must not sit
  behind bulk DMA issue, or the PSUM-pool rotation stalls the matmul
  stream and HAM down-clocks the PE. w pieces go on sync; chunk-0 x ko0
  and b on gpsimd (SWDGE, small); chunk-0 x rest upfront on scalar; later
  x pieces on scalar emitted one chunk ahead of consumption. PSUM->SBUF
  element ops split by parity between scalar ACTIVATE and vector dual-op
  TENSOR_SCALAR.
- DMA completions have a ~3-5us head latency (single hardware read queue,
  out-of-order packet completion): 26 warm-up matmuls keep the PE busy
  (and the HAM clock ramping 1.2->2.4GHz) until chunk-0 data + semaphore
  reliably land.
- y is chunk-major so every output DMA is contiguous per partition;
  non-last chunks ship as one DMA on sync; the smallest chunk runs last
  and drains per-d across engines.
"""

import sys

sys.path.insert(0, "/opt/trn_rl_repo")

import numpy as np
import ml_dtypes

BF16 = ml_dtypes.bfloat16
E, D, F, N_CORES = 8, 512, 2048, 8
KD = D // 128      # 4 contraction tiles (stage1) == output d tiles (stage2)
FS = 512           # F-columns per slot (quarter of F)
KQ = FS // 128     # 4
NSLOT = 4
N_WARM = 26

_cache: dict = {}


def _chunks_of(w: int) -> list[int]:
    # split width into <=512-col chunks (PSUM bank limit), evenly
    if w <= 512:
        return [w]
    n = -(-w // 512)
    base = (-(-w // n) + 15) // 16 * 16
    out, rem = [], w
    while rem > 0:
        c = min(base, rem)
        out.append(c)
        rem -= c
    return out


def _chunk_list(widths):
    """Chunk schedule: (slot, lo, cw, off) in execution order; the
    smallest chunk is moved to the end to shorten the drain tail."""
    ch = []
    for s, w in enumerate(widths):
        lo = 0
        for cw in _chunks_of(w):
            ch.append([s, lo, cw])
            lo += cw
    k = min(range(len(ch)), key=lambda i: (ch[i][2], -i))
    ch.append(ch.pop(k))
    off = 0
    out = []
    for s, lo, cw in ch:
        out.append((s, lo, cw, off))  # x and y share chunk-major offsets
        off += KD * cw
    return out, off


def _build(widths: tuple[int, ...]):
    import concourse.tile as tile
    import concourse.mybir as mybir
    from concourse import bacc

    f32 = mybir.dt.float32
    bf16 = mybir.dt.bfloat16
    Relu = mybir.ActivationFunctionType.Relu
    Ident = mybir.ActivationFunctionType.Identity
    Add = mybir.AluOpType.add
    Max = mybir.AluOpType.max

    nc = bacc.Bacc("TRN2", target_bir_lowering=False, debug=False)

    CH, xtot = _chunk_list(widths)
    n_ch = len(CH)

    # Layouts (all [128, *]):
    #   w[p, s*4096 + f*512 + ko*128 + c]        = W1[e][128*ko+p, 512*q + 128*f + c]
    #   w[p, s*4096 + 2048 + d*512 + fo*128 + c] = W2[e][512*q + 128*fo + p, 128*d + c]
    #   x[p, xoff + ko*cw + c]                   = x[tok_{lo+c}, 128*ko + p]
    #   b[p, s*8+f] = b1[e][512*q+128*f+p];  b[p, s*8+4+d] = b2[e][128*d+p] (q==0 else 0)
    #   y[p, yoff + d*cw + c]                    = partial y[tok_{lo+c}, 128*d+p]
    w_d = nc.dram_tensor("w", [128, NSLOT * 4096], bf16, kind="ExternalInput").ap()
    x_d = nc.dram_tensor("x", [128, xtot], bf16, kind="ExternalInput").ap()
    b_d = nc.dram_tensor("b", [128, NSLOT * 8], f32, kind="ExternalInput").ap()
    y_d = nc.dram_tensor("y", [128, xtot], bf16, kind="ExternalOutput").ap()

    # Emission plan: st1(i+1) between st1(i) and st2(i)
    plan = [("st1", 0)]
    for i in range(n_ch):
        if i + 1 < n_ch:
            plan.append(("st1", i + 1))
        plan.append(("st2", i))

    # w pieces (sync queue), in consumption order. Slot 0's w1 is split fine
    # so the first matmul group can fire early; later pieces are 1024-col
    # (256KB) to keep issue cost ahead of the ~350GB/s drain.
    wpieces = []
    wseen = set()

    def need_w(kind, s):
        if (kind, s) in wseen:
            return
        wseen.add((kind, s))
        base = s * 4096 + (0 if kind == "w1" else 2048)
        if kind == "w1" and s == 0:
            wpieces.append((base, base + 512))
            wpieces.append((base + 512, base + 1024))
            wpieces.append((base + 1024, base + 2048))
        else:
            wpieces.append((base, base + 1024))
            wpieces.append((base + 1024, base + 2048))

    for op, ci in plan:
        s = CH[ci][0]
        need_w("w1" if op == "st1" else "w2", s)

    with tile.TileContext(nc) as tc:
        with tc.tile_pool(name="wp", bufs=1) as wp, \
             tc.tile_pool(name="hp", bufs=2) as hp, \
             tc.tile_pool(name="yp", bufs=2) as yp, \
             tc.tile_pool(name="scr", bufs=1) as scr, \
             tc.tile_pool(name="pp", bufs=3, space="PSUM") as pp:

            # --- PE warm-up: dummy matmuls bridge the DMA head latency and
            # keep the HAM clock ramp going until chunk-0 data lands.
            wrm = scr.tile([128, 256], bf16, name="wrm")
            nc.gpsimd.memset(wrm[:], 0.0)
            wps = pp.tile([128, 256], f32, name="wps", tag="wps", bufs=1)
            for _ in range(N_WARM):
                nc.tensor.matmul(wps[:], wrm[:, :128], wrm[:], start=True, stop=True)

            wt = wp.tile([128, NSLOT * 4096], bf16, name="wt")
            xt = wp.tile([128, xtot], bf16, name="xt")
            bis = wp.tile([128, NSLOT * 8], f32, name="bis")

            # --- head DMA issue ---
            for lo, hi in wpieces:
                nc.sync.dma_start(wt[:, lo:hi], w_d[:, lo:hi])
            s0, lo0, cw0, off0 = CH[0]
            nc.gpsimd.dma_start(xt[:, off0: off0 + cw0], x_d[:, off0: off0 + cw0])
            nc.gpsimd.dma_start(bis[:], b_d[:])
            nc.scalar.dma_start(xt[:, off0 + cw0: off0 + KD * cw0],
                                x_d[:, off0 + cw0: off0 + KD * cw0])

            # --- compute ---
            hs = {}
            xseen = {0}

            def xload(ci):
                if ci in xseen or ci >= n_ch:
                    return
                xseen.add(ci)
                _, _, cw, off = CH[ci]
                nc.scalar.dma_start(xt[:, off: off + KD * cw],
                                    x_d[:, off: off + KD * cw])

            def st1(ci):
                xload(ci + 1)  # scalar issues next chunk's x one chunk ahead
                s, lo, cw, off = CH[ci]
                for f in range(KQ):
                    p1 = pp.tile([128, 512], f32, name=f"p1_{ci}_{f}", tag="p1")
                    for ko in range(KD):
                        lhsT = wt[:, s * 4096 + f * 512 + ko * 128:
                                  s * 4096 + f * 512 + ko * 128 + 128]
                        rhs = xt[:, off + ko * cw: off + (ko + 1) * cw]
                        nc.tensor.matmul(p1[:, :cw], lhsT, rhs,
                                         start=(ko == 0), stop=(ko == KD - 1))
                    h = hp.tile([128, 512], bf16, name=f"h{ci}_{f}", tag=f"h{f}")
                    bcol = bis[:, s * 8 + f: s * 8 + f + 1]
                    if f % 2 == 0:
                        nc.scalar.activation(h[:, :cw], p1[:, :cw], Relu, bias=bcol)
                    else:
                        nc.vector.tensor_scalar(h[:, :cw], p1[:, :cw], bcol, 0.0,
                                                Add, Max)
                    hs[(ci, f)] = h

            def st2(ci, last):
                s, lo, cw, off = CH[ci]
                ys = yp.tile([128, KD * 512], bf16, name=f"ys{ci}", tag="ys")
                for d in range(KD):
                    p2 = pp.tile([128, 512], f32, name=f"p2_{ci}_{d}",
                                 tag=f"p2_{d}", bufs=1)
                    for fo in range(KQ):
                        lhsT = wt[:, s * 4096 + 2048 + d * 512 + fo * 128:
                                  s * 4096 + 2048 + d * 512 + fo * 128 + 128]
                        nc.tensor.matmul(p2[:, :cw], lhsT, hs[(ci, fo)][:, :cw],
                                         start=(fo == 0), stop=(fo == KQ - 1))
                    bcol = bis[:, s * 8 + 4 + d: s * 8 + 4 + d + 1]
                    if d % 2 == 0:
                        nc.scalar.activation(ys[:, d * cw:(d + 1) * cw],
                                             p2[:, :cw], Ident, bias=bcol)
                    else:
                        nc.vector.tensor_scalar_add(ys[:, d * cw:(d + 1) * cw],
                                                    p2[:, :cw], bcol)
                    if last:
                        eng = [nc.gpsimd, nc.scalar, nc.gpsimd, nc.sync][d]
                        eng.dma_start(y_d[:, off + d * cw: off + (d + 1) * cw],
                                      ys[:, d * cw:(d + 1) * cw])
                if not last:
                    nc.sync.dma_start(y_d[:, off: off + KD * cw],
                                      ys[:, :KD * cw])

            for op, ci in plan:
                if op == "st1":
                    st1(ci)
                else:
                    st2(ci, last=(ci == n_ch - 1))

    nc.compile()
    return nc


def _get_nc(widths: tuple[int, ...]):
    if widths not in _cache:
        _cache[widths] = _build(widths)
    return _cache[widths]


def _plan(counts):
    """Pair adjacent experts in sorted order into NSLOT slots (minimizes
    sum of per-slot maxima); return (pairs, widths)."""
    order = np.argsort(-counts, kind="stable")
    pairs = [(int(order[2 * s]), int(order[2 * s + 1])) for s in range(NSLOT)]
    widths = tuple(
        (max(int(counts[a]), int(counts[b]), 16) + 15) // 16 * 16
        for a, b in pairs)
    return pairs, widths


def _pack_inputs(x, W1, b1, W2, b2, order, starts, pairs, widths):
    """Build per-core in_maps. Core j, slot s: expert pair[s][j//4], quarter j%4."""
    CH, xtot = _chunk_list(widths)
    xbf = x.astype(BF16)
    toks_of = [order[starts[e]:starts[e + 1]] for e in range(E)]
    in_maps = []
    for j in range(N_CORES):
        q = j % 4
        wcols = np.empty((128, NSLOT * 4096), BF16)
        bcols = np.zeros((128, NSLOT * 8), np.float32)
        xcols = np.zeros((128, xtot), BF16)
        xe_cache = {}
        for s in range(NSLOT):
            e = pairs[s][0] if j < 4 else pairs[s][1]
            # w1 (f-major): [p, f*512 + ko*128 + c]
            w1s = W1[e][:, FS * q: FS * (q + 1)]               # [D, Fs]
            wcols[:, s * 4096: s * 4096 + 2048] = \
                w1s.reshape(KD, 128, KQ, 128).transpose(1, 2, 0, 3).reshape(128, KD * FS)
            # w2 (d-major): [p, d*512 + fo*128 + c]
            w2s = W2[e][FS * q: FS * (q + 1), :]               # [Fs, D]
            wcols[:, s * 4096 + 2048: s * 4096 + 4096] = \
                w2s.reshape(KQ, 128, KD, 128).transpose(1, 2, 0, 3).reshape(128, KQ * D)
            bcols[:, s * 8: s * 8 + KQ] = b1[e][FS * q: FS * (q + 1)].reshape(KQ, 128).T
            if q == 0:
                bcols[:, s * 8 + 4: s * 8 + 8] = b2[e].reshape(KD, 128).T
            toks = toks_of[e]
            xe = np.zeros((widths[s], D), BF16)
            xe[:len(toks)] = xbf[toks]
            xe_cache[s] = xe.T                                  # [D, W]
        for s, lo, cw, off in CH:
            xcols[:, off: off + KD * cw] = \
                xe_cache[s][:, lo:lo + cw].reshape(KD, 128, cw) \
                .transpose(1, 0, 2).reshape(128, KD * cw)
        in_maps.append({
            "w": np.ascontiguousarray(wcols),
            "x": np.ascontiguousarray(xcols),
            "b": bcols,
        })
    return in_maps, toks_of


def kernel(x, Wg, bg, W1, b1, W2, b2):
    from concourse.bass_utils import run_bass_kernel_spmd

    x = np.asarray(x, dtype=np.float32)
    n_tok = x.shape[0]

    # host gate in f64: the mathematically-true argmax
    logits = x.astype(np.float64) @ np.asarray(Wg, np.float64) + np.asarray(bg, np.float64)
    idx = logits.argmax(1)

    counts = np.bincount(idx, minlength=E)
    order = np.argsort(idx, kind="stable")
    starts = np.zeros(E + 1, np.int64)
    starts[1:] = np.cumsum(counts)

    pairs, widths = _plan(counts)

    W1 = np.asarray(W1, np.float32)
    W2 = np.asarray(W2, np.float32)
    b1 = np.asarray(b1, np.float32)
    b2 = np.asarray(b2, np.float32)

    in_maps, toks_of = _pack_inputs(x, W1, b1, W2, b2, order, starts, pairs, widths)
    nc = _get_nc(widths)
    res = run_bass_kernel_spmd(nc, in_maps, core_ids=list(range(N_CORES)))

    CH, xtot = _chunk_list(widths)
    out = np.zeros((n_tok, D), np.float32)
    for j in range(N_CORES):
        yv = res.results[j]["y"]
        for s, lo, cw, off in CH:
            e = pairs[s][0] if j < 4 else pairs[s][1]
            toks = toks_of[e]
            seg = toks[lo:lo + cw]
            if len(seg) == 0:
                continue
            blk = yv[:, off: off + KD * cw].astype(np.float32) \
                .reshape(128, KD, cw).transpose(2, 1, 0).reshape(cw, D)
            out[seg] += blk[:len(seg)]
    return out


# revision 9
# speedup vs baseline: 1.1217x; 1.0191x over previous
"""MoE top-1 routing kernel for Trainium2 (8 NeuronCores, expert-F-sharded).

Model (E=8, D=512, F=2048, N=4096):
    logits = x @ Wg + bg; e = argmax(logits)
    y[i] = relu(x[i] @ W1[e] + b1[e]) @ W2[e] + b2[e]

Strategy (v6 — quarter-F expert pairing + queue re-routing):
- Host computes the gate (f64 matmul + argmax) and routes tokens.
- Each expert's FFN is split into 4 F-quarters (Fs=512). Experts are paired
  (adjacent in sorted-count order) into 4 "slots"; slot s appears on every
  core with the same compile-time token width W_s = max count over the
  slot's two experts. Core j, slot s holds (expert = pair[s][j//4],
  quarter q = j%4) and processes ALL of that expert's tokens against its
  F-quarter. PE work per core = 32*sum_s W_s cycles; adjacent-pairing
  minimizes sum of pair maxima, so expert imbalance costs only ~4%.
- Everything on the wire is bf16; PSUM accumulates fp32; b1 is applied in
  the Relu, b2 is added only by the q==0 cell (zeros elsewhere), partial
  y's are summed on the host in fp32.
- Queue routing (the v5 lesson): the scalar/vector queues must not sit
  behind bulk DMA issue, or the PSUM-pool rotation stalls the matmul
  stream and HAM down-clocks the PE. w pieces go on sync; chunk-0 x ko0
  and b on gpsimd (SWDGE, small); chunk-0 x rest upfront on scalar; later
  x pieces on scalar emitted one chunk ahead of consumption. PSUM->SBUF
  element ops split by parity between scalar ACTIVATE and vector dual-op
  TENSOR_SCALAR.
- DMA completions have a ~3-5us head latency (single hardware read queue,
  out-of-order packet completion): 26 warm-up matmuls keep the PE busy
  (and the HAM clock ramping 1.2->2.4GHz) until chunk-0 data + semaphore
  reliably land.
- y is chunk-major so every output DMA is contiguous per partition;
  non-last chunks ship as one DMA on sync; the smallest chunk runs last
  and drains per-d across engines.
"""

import sys

sys.path.insert(0, "/opt/trn_rl_repo")

import numpy as np
import ml_dtypes

BF16 = ml_dtypes.bfloat16
E, D, F, N_CORES = 8, 512, 2048, 8
KD = D // 128      # 4 contraction tiles (stage1) == output d tiles (stage2)
FS = 512           # F-columns per slot (quarter of F)
KQ = FS // 128     # 4
NSLOT = 4
N_WARM = 22

_cache: dict = {}


def _chunks_of(w: int) -> list[int]:
    # split width into <=512-col chunks (PSUM bank limit), evenly
    if w <= 512:
        return [w]
    n = -(-w // 512)
    base = (-(-w // n) + 15) // 16 * 16
    out, rem = [], w
    while rem > 0:
        c = min(base, rem)
        out.append(c)
        rem -= c
    return out


def _chunk_list(widths):
    """Chunk schedule: (slot, lo, cw, off) in execution order; the
    smallest chunk is moved to the end to shorten the drain tail."""
    ch = []
    for s, w in enumerate(widths):
        lo = 0
        for cw in _chunks_of(w):
            ch.append([s, lo, cw])
            lo += cw
    k = min(range(len(ch)), key=lambda i: (ch[i][2], -i))
    ch.append(ch.pop(k))
    off = 0
    out = []
    for s, lo, cw in ch:
        out.append((s, lo, cw, off))  # x and y share chunk-major offsets
        off += KD * cw
    return out, off


def _build(widths: tuple[int, ...]):
    import concourse.tile as tile
    import concourse.mybir as mybir
    from concourse import bacc

    f32 = mybir.dt.float32
    bf16 = mybir.dt.bfloat16
    Relu = mybir.ActivationFunctionType.Relu
    Ident = mybir.ActivationFunctionType.Identity
    Add = mybir.AluOpType.add
    Max = mybir.AluOpType.max

    nc = bacc.Bacc("TRN2", target_bir_lowering=False, debug=False)

    CH, xtot = _chunk_list(widths)
    n_ch = len(CH)

    # Layouts (all [128, *]):
    #   w[p, s*4096 + f*512 + ko*128 + c]        = W1[e][128*ko+p, 512*q + 128*f + c]
    #   w[p, s*4096 + 2048 + d*512 + fo*128 + c] = W2[e][512*q + 128*fo + p, 128*d + c]
    #   x[p, xoff + ko*cw + c]                   = x[tok_{lo+c}, 128*ko + p]
    #   b[p, s*8+f] = b1[e][512*q+128*f+p];  b[p, s*8+4+d] = b2[e][128*d+p] (q==0 else 0)
    #   y[p, yoff + d*cw + c]                    = partial y[tok_{lo+c}, 128*d+p]
    w_d = nc.dram_tensor("w", [128, NSLOT * 4096], bf16, kind="ExternalInput").ap()
    x_d = nc.dram_tensor("x", [128, xtot], bf16, kind="ExternalInput").ap()
    b_d = nc.dram_tensor("b", [128, NSLOT * 8], f32, kind="ExternalInput").ap()
    y_d = nc.dram_tensor("y", [128, xtot], bf16, kind="ExternalOutput").ap()

    # Emission plan: st1(i+1) between st1(i) and st2(i)
    plan = [("st1", 0)]
    for i in range(n_ch):
        if i + 1 < n_ch:
            plan.append(("st1", i + 1))
        plan.append(("st2", i))

    # Input pieces, all on the sync queue in consumption order. The HWDGE
    # completion-semaphore pool holds only ~8 DMAs; beyond that the compiler
    # recycles sems and upgrades consumer waits to LATER pieces on the FIFO
    # ring (false dependencies). So: few, large pieces — slot 0's w split
    # fine for an early first matmul, later slots as one 1MiB piece, x
    # merged across adjacent chunks. pieces = ("w"|"x"|"b", lo, hi).
    s0 = CH[0][0]
    pieces = [("w", s0 * 4096, s0 * 4096 + 512),
              ("b", 0, NSLOT * 8),
              ("x", CH[0][3], CH[0][3] + KD * CH[0][2]),
              ("w", s0 * 4096 + 512, s0 * 4096 + 2048),
              ("w", s0 * 4096 + 2048, s0 * 4096 + 4096)]
    wseen = {s0}
    ci = 1
    while ci < n_ch:
        grp = [ci]
        # extend the x-group up to (and including) the next new-slot chunk
        while CH[grp[-1]][0] in wseen and grp[-1] + 1 < n_ch:
            grp.append(grp[-1] + 1)
        new_s = CH[grp[-1]][0]
        new_s = None if new_s in wseen else new_s
        # absorb trailing chunks whose weights are already covered
        while grp[-1] + 1 < n_ch and \
                CH[grp[-1] + 1][0] in (wseen | {new_s}):
            grp.append(grp[-1] + 1)
        last = CH[grp[-1]]
        pieces.append(("x", CH[grp[0]][3], last[3] + KD * last[2]))
        if new_s is not None:
            wseen.add(new_s)
            pieces.append(("w", new_s * 4096, new_s * 4096 + 4096))
        ci = grp[-1] + 1

    with tile.TileContext(nc) as tc:
        with tc.tile_pool(name="wp", bufs=1) as wp, \
             tc.tile_pool(name="hp", bufs=2) as hp, \
             tc.tile_pool(name="yp", bufs=2) as yp, \
             tc.tile_pool(name="scr", bufs=1) as scr, \
             tc.tile_pool(name="pp", bufs=3, space="PSUM") as pp:

            # --- PE warm-up: dummy matmuls bridge the DMA head latency and
            # keep the HAM clock ramp going until chunk-0 data lands.
            wrm = scr.tile([128, 256], bf16, name="wrm")
            nc.gpsimd.memset(wrm[:], 0.0)
            wps = pp.tile([128, 256], f32, name="wps", tag="wps", bufs=1)
            for _ in range(N_WARM):
                nc.tensor.matmul(wps[:], wrm[:, :128], wrm[:], start=True, stop=True)

            wt = wp.tile([128, NSLOT * 4096], bf16, name="wt")
            xt = wp.tile([128, xtot], bf16, name="xt")
            bis = wp.tile([128, NSLOT * 8], f32, name="bis")

            # --- head DMA issue: everything on sync, consumption order ---
            for kind, lo, hi in pieces:
                t, dr = {"w": (wt, w_d), "x": (xt, x_d), "b": (bis, b_d)}[kind]
                nc.sync.dma_start(t[:, lo:hi], dr[:, lo:hi])

            # --- compute ---
            hs = {}

            def st1(ci):
                s, lo, cw, off = CH[ci]
                for f in range(KQ):
                    p1 = pp.tile([128, 512], f32, name=f"p1_{ci}_{f}", tag="p1")
                    for ko in range(KD):
                        lhsT = wt[:, s * 4096 + f * 512 + ko * 128:
                                  s * 4096 + f * 512 + ko * 128 + 128]
                        rhs = xt[:, off + ko * cw: off + (ko + 1) * cw]
                        nc.tensor.matmul(p1[:, :cw], lhsT, rhs,
                                         start=(ko == 0), stop=(ko == KD - 1))
                    h = hp.tile([128, 512], bf16, name=f"h{ci}_{f}", tag=f"h{f}")
                    bcol = bis[:, s * 8 + f: s * 8 + f + 1]
                    if f % 2 == 0:
                        nc.scalar.activation(h[:, :cw], p1[:, :cw], Relu, bias=bcol)
                    else:
                        nc.vector.tensor_scalar(h[:, :cw], p1[:, :cw], bcol, 0.0,
                                                Add, Max)
                    hs[(ci, f)] = h

            def st2(ci, last):
                s, lo, cw, off = CH[ci]
                ys = yp.tile([128, KD * 512], bf16, name=f"ys{ci}", tag="ys")
                for d in range(KD):
                    p2 = pp.tile([128, 512], f32, name=f"p2_{ci}_{d}",
                                 tag=f"p2_{d}", bufs=1)
                    for fo in range(KQ):
                        lhsT = wt[:, s * 4096 + 2048 + d * 512 + fo * 128:
                                  s * 4096 + 2048 + d * 512 + fo * 128 + 128]
                        nc.tensor.matmul(p2[:, :cw], lhsT, hs[(ci, fo)][:, :cw],
                                         start=(fo == 0), stop=(fo == KQ - 1))
                    bcol = bis[:, s * 8 + 4 + d: s * 8 + 4 + d + 1]
                    if d % 2 == 0:
                        nc.scalar.activation(ys[:, d * cw:(d + 1) * cw],
                                             p2[:, :cw], Ident, bias=bcol)
                    else:
                        nc.vector.tensor_scalar_add(ys[:, d * cw:(d + 1) * cw],
                                                    p2[:, :cw], bcol)
                    if last:
                        eng = [nc.sync, nc.scalar, nc.sync, nc.scalar][d]
                        eng.dma_start(y_d[:, off + d * cw: off + (d + 1) * cw],
                                      ys[:, d * cw:(d + 1) * cw])
                if not last:
                    nc.sync.dma_start(y_d[:, off: off + KD * cw],
                                      ys[:, :KD * cw])

            for op, ci in plan:
                if op == "st1":
                    st1(ci)
                else:
                    st2(ci, last=(ci == n_ch - 1))

    nc.compile()
    return nc


def _get_nc(widths: tuple[int, ...]):
    if widths not in _cache:
        _cache[widths] = _build(widths)
    return _cache[widths]


def _plan(counts):
    """Pair adjacent experts in sorted order into NSLOT slots (minimizes
    sum of per-slot maxima); return (pairs, widths)."""
    order = np.argsort(-counts, kind="stable")
    pairs = [(int(order[2 * s]), int(order[2 * s + 1])) for s in range(NSLOT)]
    widths = tuple(
        (max(int(counts[a]), int(counts[b]), 16) + 15) // 16 * 16
        for a, b in pairs)
    return pairs, widths


def _pack_inputs(x, W1, b1, W2, b2, order, starts, pairs, widths):
    """Build per-core in_maps. Core j, slot s: expert pair[s][j//4], quarter j%4."""
    CH, xtot = _chunk_list(widths)
    xbf = x.astype(BF16)
    toks_of = [order[starts[e]:starts[e + 1]] for e in range(E)]
    in_maps = []
    for j in range(N_CORES):
        q = j % 4
        wcols = np.empty((128, NSLOT * 4096), BF16)
        bcols = np.zeros((128, NSLOT * 8), np.float32)
        xcols = np.zeros((128, xtot), BF16)
        xe_cache = {}
        for s in range(NSLOT):
            e = pairs[s][0] if j < 4 else pairs[s][1]
            # w1 (f-major): [p, f*512 + ko*128 + c]
            w1s = W1[e][:, FS * q: FS * (q + 1)]               # [D, Fs]
            wcols[:, s * 4096: s * 4096 + 2048] = \
                w1s.reshape(KD, 128, KQ, 128).transpose(1, 2, 0, 3).reshape(128, KD * FS)
            # w2 (d-major): [p, d*512 + fo*128 + c]
            w2s = W2[e][FS * q: FS * (q + 1), :]               # [Fs, D]
            wcols[:, s * 4096 + 2048: s * 4096 + 4096] = \
                w2s.reshape(KQ, 128, KD, 128).transpose(1, 2, 0, 3).reshape(128, KQ * D)
            bcols[:, s * 8: s * 8 + KQ] = b1[e][FS * q: FS * (q + 1)].reshape(KQ, 128).T
            if q == 0:
                bcols[:, s * 8 + 4: s * 8 + 8] = b2[e].reshape(KD, 128).T
            toks = toks_of[e]
            xe = np.zeros((widths[s], D), BF16)
            xe[:len(toks)] = xbf[toks]
            xe_cache[s] = xe.T                                  # [D, W]
        for s, lo, cw, off in CH:
            xcols[:, off: off + KD * cw] = \
                xe_cache[s][:, lo:lo + cw].reshape(KD, 128, cw) \
                .transpose(1, 0, 2).reshape(128, KD * cw)
        in_maps.append({
            "w": np.ascontiguousarray(wcols),
            "x": np.ascontiguousarray(xcols),
            "b": bcols,
        })
    return in_maps, toks_of


def kernel(x, Wg, bg, W1, b1, W2, b2):
    from concourse.bass_utils import run_bass_kernel_spmd

    x = np.asarray(x, dtype=np.float32)
    n_tok = x.shape[0]

    # host gate in f64: the mathematically-true argmax
    logits = x.astype(np.float64) @ np.asarray(Wg, np.float64) + np.asarray(bg, np.float64)
    idx = logits.argmax(1)

    counts = np.bincount(idx, minlength=E)
    order = np.argsort(idx, kind="stable")
    starts = np.zeros(E + 1, np.int64)
    starts[1:] = np.cumsum(counts)

    pairs, widths = _plan(counts)

    W1 = np.asarray(W1, np.float32)
    W2 = np.asarray(W2, np.float32)
    b1 = np.asarray(b1, np.float32)
    b2 = np.asarray(b2, np.float32)

    in_maps, toks_of = _pack_inputs(x, W1, b1, W2, b2, order, starts, pairs, widths)
    nc = _get_nc(widths)
    res = run_bass_kernel_spmd(nc, in_maps, core_ids=list(range(N_CORES)))

    CH, xtot = _chunk_list(widths)
    out = np.zeros((n_tok, D), np.float32)
    for j in range(N_CORES):
        yv = res.results[j]["y"]
        for s, lo, cw, off in CH:
            e = pairs[s][0] if j < 4 else pairs[s][1]
            toks = toks_of[e]
            seg = toks[lo:lo + cw]
            if len(seg) == 0:
                continue
            blk = yv[:, off: off + KD * cw].astype(np.float32) \
                .reshape(128, KD, cw).transpose(2, 1, 0).reshape(cw, D)
            out[seg] += blk[:len(seg)]
    return out
